# revision 1
# baseline (speedup 1.0000x reference)
"""Trainium2 Bass kernel for nn_BidirectionalBoxPool.

Contract: kernel(x, boxes) takes FULL inputs (x: (8,128,128,128) f32,
boxes: (8,64,4) f32) and returns (feats, widths) matching the reference:
feats (8, 64, 2, 128, 8, MW) f32, widths (8, 64, 2) f32, with MW the
data-dependent max pooled width.

Strategy: data-parallel over the batch axis N — core n handles image n.

Math per image: grid_sample with a per-box separable bilinear grid, so
  feats[k,d,c,i,j] = sum_h sum_w img[c,h,w] * wy_k[h,i] * wx_k[w,j']
with the dir-1 grid being an exact (i,j)-flip of dir-0 within each box's
valid width. Host code (numpy) replicates the reference's fp32 grid math
exactly and bakes it into two small weight tensors per image:
  WY  [h=128, K*8]     y-interp weights (f32, fed to fp32r matmuls)
  WXF [w=128, K*2*MW]  x-interp weights, dir0 block + flipped dir1 block (f16)
Device program (SPMD identical across cores; all box data flows through
the weight tensors):
  stage 1: per channel c: PSUM[w,(k,i)] = img_c[h,w]^T @ WY   (fp32r)
           -> SBUF S[w, (k,i)*C+c] in fp16
  stage 2: per (k,i): PSUM2[c, cols] = S_ki[w,c]^T @ WXF_k    (fp16)
           two matmuls (dir0/dir1) into bank-aligned PSUM halves
  out: per box, copy PSUM2 -> SBUF [c,(d,i,j)] and DMA to DRAM feats[k].
Tall boxes (bh > bw, ~7%, width<=16) have their grid transposed relative
to the wide layout; they are zeroed on device (zero weights) and patched
in exactly on host, as is the `widths` output.
"""

from contextlib import ExitStack

import numpy as np

import concourse.bass as bass
import concourse.tile as tile
from concourse import bacc, bass_utils, mybir

F32 = mybir.dt.float32
F32R = mybir.dt.float32r
F16 = mybir.dt.float16

PH = 8
N_CORES = 8
NPF32 = np.float32


# ----------------------------------------------------------------------------
# host-side weight construction (replicates reference fp32 grid math)
# ----------------------------------------------------------------------------

def _box_meta(boxes):
    b = boxes.astype(NPF32)
    xmin, ymin, xmax, ymax = b[:, 0], b[:, 1], b[:, 2], b[:, 3]
    valid = ~((xmin == 0) & (ymin == 0) & (xmax == 0) & (ymax == 0))
    one = NPF32(1.0)
    bw = np.where(valid, (xmax - xmin).astype(NPF32), one).astype(NPF32)
    bh = np.where(valid, (ymax - ymin).astype(NPF32), one).astype(NPF32)
    wide = bw > bh
    ratio = np.where(wide, (bw / bh).astype(NPF32),
                     (bh / bw).astype(NPF32)).astype(NPF32)
    width = np.ceil((ratio * NPF32(PH)).astype(NPF32)).astype(np.int32)
    width = np.where(valid, width, 0)
    wf = np.maximum(width, 2).astype(NPF32)
    return valid, wide, width, wf, bw, bh, xmin.astype(NPF32), ymin.astype(NPF32)


def _max_width(boxes_all):
    b = np.asarray(boxes_all, dtype=np.float64)
    valid = ~np.all(b == 0, axis=-1)
    bw = np.where(valid, b[..., 2] - b[..., 0], 1.0)
    bh = np.where(valid, b[..., 3] - b[..., 1], 1.0)
    ratio = np.where(bw > bh, bw / bh, bh / bw)
    ratio = np.where(valid, ratio, 0.0)
    return int(np.ceil(ratio.max() * PH))


def _grid_wide(xmin, ymin, bw, bh, wf, W, H, ii, jj):
    gx = ((xmin + (jj * bw / (wf - NPF32(1.0))).astype(NPF32)).astype(NPF32)
          - NPF32(W / 2)) / NPF32(W / 2)
    gy = ((ymin + (ii * bh / NPF32(PH - 1.0)).astype(NPF32)).astype(NPF32)
          - NPF32(H / 2)) / NPF32(H / 2)
    return gx.astype(NPF32), gy.astype(NPF32)


def _grid_tall(xmin, ymin, bw, bh, wf, W, H, ii, jj):
    gx = ((xmin + (ii * bw / NPF32(PH - 1.0)).astype(NPF32)).astype(NPF32)
          - NPF32(W / 2)) / NPF32(W / 2)
    gy = ((ymin + ((wf - jj) * bh / (wf - NPF32(1.0))).astype(NPF32)).astype(NPF32)
          - NPF32(H / 2)) / NPF32(H / 2)
    return gx.astype(NPF32), gy.astype(NPF32)


def _taps(g, n):
    g = g.astype(NPF32)
    pos = ((g + NPF32(1.0)) * NPF32(n) - NPF32(1.0)) * NPF32(0.5)
    pos64 = pos.astype(np.float64)
    i0 = np.floor(pos64).astype(np.int64)
    f = pos64 - i0
    w0 = np.where((i0 >= 0) & (i0 <= n - 1), 1.0 - f, 0.0)
    w1 = np.where((i0 + 1 >= 0) & (i0 + 1 <= n - 1), f, 0.0)
    return i0, w0, w1


def _build_image_weights(boxes, H, W, MW):
    K = boxes.shape[0]
    valid, wide, width, wf, bw, bh, xmin, ymin = _box_meta(boxes)
    WY = np.zeros((H, K * PH), np.float64)
    WXF = np.zeros((W, K * 2 * MW), np.float64)
    tall_idx = []
    ii = np.arange(PH, dtype=NPF32)
    for k in range(K):
        if not valid[k]:
            continue
        if not wide[k]:
            tall_idx.append(k)
            continue
        wk = int(width[k])
        jj = np.arange(wk, dtype=NPF32)
        gx, gy = _grid_wide(xmin[k], ymin[k], bw[k], bh[k], wf[k], W, H, ii, jj)
        y0, wy0, wy1 = _taps(gy, H)
        for i in range(PH):
            col = k * PH + i
            if wy0[i] != 0.0:
                WY[y0[i], col] += wy0[i]
            if wy1[i] != 0.0:
                WY[y0[i] + 1, col] += wy1[i]
        x0, wx0, wx1 = _taps(gx, W)
        base = k * 2 * MW
        for j in range(min(wk, MW)):
            if wx0[j] != 0.0:
                WXF[x0[j], base + j] += wx0[j]
            if wx1[j] != 0.0:
                WXF[x0[j] + 1, base + j] += wx1[j]
            jr = wk - 1 - j
            if wx0[jr] != 0.0:
                WXF[x0[jr], base + MW + j] += wx0[jr]
            if wx1[jr] != 0.0:
                WXF[x0[jr] + 1, base + MW + j] += wx1[jr]
    return WY.astype(NPF32), WXF.astype(NPF32), width, tall_idx


def _tall_feats(img, boxes, k, H, W, MW):
    valid, wide, width, wf, bw, bh, xmin, ymin = _box_meta(boxes)
    C = img.shape[0]
    wk = int(width[k])
    out = np.zeros((2, C, PH, MW), NPF32)
    ii = np.arange(PH, dtype=NPF32)[:, None]
    jj = np.arange(wk, dtype=NPF32)[None, :]
    gx, gy = _grid_tall(xmin[k], ymin[k], bw[k], bh[k], wf[k], W, H, ii, jj)
    gx = np.broadcast_to(gx, (PH, wk))
    gy = np.broadcast_to(gy, (PH, wk))
    x0, wx0, wx1 = _taps(gx, W)
    y0, wy0, wy1 = _taps(gy, H)
    imgf = img.astype(np.float64)

    def gat(yc, xc, m):
        yi = np.clip(yc, 0, H - 1)
        xi = np.clip(xc, 0, W - 1)
        return imgf[:, yi, xi] * m

    s = (gat(y0, x0, wy0 * wx0) + gat(y0, x0 + 1, wy0 * wx1)
         + gat(y0 + 1, x0, wy1 * wx0) + gat(y0 + 1, x0 + 1, wy1 * wx1))
    wcl = min(wk, MW)
    out[0, :, :, :wcl] = s[:, :, :wcl].astype(NPF32)
    out[1, :, :, :wcl] = s[:, ::-1, ::-1][:, :, :wcl].astype(NPF32)
    return out


# ----------------------------------------------------------------------------
# device program
# ----------------------------------------------------------------------------

def _build_program(C, K, MW, KG=32, P=128):
    NW = 2 * MW
    G = K // KG
    assert K % KG == 0

    nc = bacc.Bacc("TRN2", target_bir_lowering=False, debug=False,
                   enable_asserts=True, num_devices=1)

    img = nc.dram_tensor("img", [P, C * P], F32R, kind="ExternalInput").ap()
    wy = nc.dram_tensor("wy", [P, K * PH], F32R, kind="ExternalInput").ap()
    wxf = nc.dram_tensor("wxf", [P, K * NW], F16, kind="ExternalInput").ap()
    feats = nc.dram_tensor("feats", [K, 2 * C * PH * MW], F32,
                           kind="ExternalOutput").ap()

    blk = PH * MW
    off1 = ((blk + 511) // 512) * 512  # dir1 at a PSUM-bank-aligned column
    ps2_cols = off1 + blk

    with tile.TileContext(nc) as tc, ExitStack() as ctx:
        const_pool = ctx.enter_context(tc.tile_pool(name="const", bufs=1))
        s_pool = ctx.enter_context(tc.tile_pool(name="sg", bufs=1))
        out_pool = ctx.enter_context(tc.tile_pool(name="outt", bufs=3))
        ps1_pool = ctx.enter_context(tc.tile_pool(name="ps1", bufs=4, space="PSUM"))
        ps2_pool = ctx.enter_context(tc.tile_pool(name="ps2", bufs=2, space="PSUM"))

        img_t = const_pool.tile([P, C * P], F32R)
        wy_t = const_pool.tile([P, K * PH], F32R)
        wxf_t = const_pool.tile([P, K * NW], F16)
        nc.sync.dma_start(img_t[:], img)
        nc.sync.dma_start(wy_t[:], wy)
        nc.sync.dma_start(wxf_t[:], wxf)

        for g in range(G):
            sg = s_pool.tile([P, KG * PH * C], F16, tag="sg")
            ncols = KG * PH
            for c in range(C):
                ps1 = ps1_pool.tile([P, ncols], F32, tag="ps1")
                nc.tensor.matmul(
                    ps1[:],
                    img_t[:, c * P:(c + 1) * P],
                    wy_t[:, g * ncols:(g + 1) * ncols],
                )
                nc.vector.tensor_copy(sg[:, c::C], ps1[:])

            for kl in range(KG):
                k = g * KG + kl
                ps2 = ps2_pool.tile([C, ps2_cols], F32, tag="ps2")
                for i in range(PH):
                    lhsT = sg[:, (kl * PH + i) * C:(kl * PH + i + 1) * C]
                    nc.tensor.matmul(
                        ps2[:, i * MW:(i + 1) * MW],
                        lhsT, wxf_t[:, k * NW:k * NW + MW])
                    nc.tensor.matmul(
                        ps2[:, off1 + (PH - 1 - i) * MW:off1 + (PH - i) * MW],
                        lhsT, wxf_t[:, k * NW + MW:k * NW + 2 * MW])

                outt = out_pool.tile([C, 2 * blk], F32, tag="outt")
                nc.vector.tensor_copy(outt[:, :blk], ps2[:, :blk])
                nc.vector.tensor_copy(outt[:, blk:], ps2[:, off1:off1 + blk])
                # DRAM box block order is (d, c, i, j); SBUF iterates (c, d, ij)
                box = bass.AP(feats.tensor, k * 2 * C * blk,
                              [[blk, C], [C * blk, 2], [1, blk]])
                nc.sync.dma_start(box, outt[:])

    nc.compile()
    return nc


_PROGRAM_CACHE = {}


def _get_program(C, K, MW):
    key = (C, K, MW)
    if key not in _PROGRAM_CACHE:
        _PROGRAM_CACHE[key] = _build_program(C, K, MW)
    return _PROGRAM_CACHE[key]


# ----------------------------------------------------------------------------
# entry point
# ----------------------------------------------------------------------------

def kernel(x, boxes, _run_kwargs=None):
    x = np.asarray(x, dtype=np.float32)
    boxes = np.asarray(boxes, dtype=np.float32)
    N, C, H, W = x.shape
    K = boxes.shape[1]
    assert N == N_CORES and H == 128 and W == 128 and C == 128

    MW = _max_width(boxes)
    nc = _get_program(C, K, MW)

    in_maps = []
    per_image = []
    for n in range(N):
        WY, WXF, width, tall_idx = _build_image_weights(boxes[n], H, W, MW)
        per_image.append((width, tall_idx))
        img = np.ascontiguousarray(
            x[n].transpose(1, 0, 2).reshape(H, C * W))  # [h, (c, w)]
        in_maps.append({
            "img": img,
            "wy": WY,
            "wxf": WXF.astype(np.float16),
        })

    res = bass_utils.run_bass_kernel_spmd(
        nc, in_maps, core_ids=list(range(N_CORES)), **(_run_kwargs or {}))

    feats = np.empty((N, K, 2, C, PH, MW), np.float32)
    widths = np.empty((N, K, 2), np.float32)
    for n in range(N):
        feats[n] = res.results[n]["feats"].reshape(K, 2, C, PH, MW)
        width, tall_idx = per_image[n]
        for k in tall_idx:
            feats[n, k] = _tall_feats(x[n], boxes[n], k, H, W, MW)
        widths[n] = width.astype(np.float32)[:, None]
    kernel.last_result = res
    return feats, widths


# revision 5
# speedup vs baseline: 1.7377x; 1.7377x over previous
"""Trainium2 Bass kernel for nn_BidirectionalBoxPool.

Contract: kernel(x, boxes) takes FULL inputs (x: (8,128,128,128) f32,
boxes: (8,64,4) f32) and returns (feats, widths) matching the reference:
feats (8, 64, 2, 128, 8, MW) f32, widths (8, 64, 2) f32, with MW the
data-dependent max pooled width.

Strategy: data-parallel over the batch axis N — core n handles image n.

Math per image: grid_sample with a per-box separable bilinear grid, so
  feats[k,d,c,i,j] = sum_h sum_w img[c,h,w] * wy_k[h,i] * wx_k[w,j']
with the dir-1 grid being an exact (i,j)-flip of dir-0 within each box's
valid width. Host code (numpy) replicates the reference's fp32 grid math
exactly and bakes it into two small weight tensors per image:
  WY  [h=128, K*8]     y-interp weights (f32, fed to fp32r matmuls)
  WXF [w=128, K*2*MW]  x-interp weights, dir0 block + flipped dir1 block (f16)
Device program (SPMD identical across cores; all box data flows through
the weight tensors):
  stage 1: per channel c: PSUM[w,(k,i)] = img_c[h,w]^T @ WY   (fp32r)
           -> SBUF S[w, (k,i)*C+c] in fp16
  stage 2: per (k,i): PSUM2[c, cols] = S_ki[w,c]^T @ WXF_k    (fp16)
           two matmuls (dir0/dir1) into bank-aligned PSUM halves
  out: per box, copy PSUM2 -> SBUF [c,(d,i,j)] and DMA to DRAM feats[k].
Tall boxes (bh > bw, ~7%, width<=16) have their grid transposed relative
to the wide layout; they are zeroed on device (zero weights) and patched
in exactly on host, as is the `widths` output.
"""

from contextlib import ExitStack

import numpy as np

import concourse.bass as bass
import concourse.tile as tile
from concourse import bacc, bass_utils, mybir

F32 = mybir.dt.float32
F32R = mybir.dt.float32r
F16 = mybir.dt.float16

PH = 8
N_CORES = 8
NPF32 = np.float32


# ----------------------------------------------------------------------------
# host-side weight construction (replicates reference fp32 grid math)
# ----------------------------------------------------------------------------

def _box_meta(boxes):
    b = boxes.astype(NPF32)
    xmin, ymin, xmax, ymax = b[:, 0], b[:, 1], b[:, 2], b[:, 3]
    valid = ~((xmin == 0) & (ymin == 0) & (xmax == 0) & (ymax == 0))
    one = NPF32(1.0)
    bw = np.where(valid, (xmax - xmin).astype(NPF32), one).astype(NPF32)
    bh = np.where(valid, (ymax - ymin).astype(NPF32), one).astype(NPF32)
    wide = bw > bh
    ratio = np.where(wide, (bw / bh).astype(NPF32),
                     (bh / bw).astype(NPF32)).astype(NPF32)
    width = np.ceil((ratio * NPF32(PH)).astype(NPF32)).astype(np.int32)
    width = np.where(valid, width, 0)
    wf = np.maximum(width, 2).astype(NPF32)
    return valid, wide, width, wf, bw, bh, xmin.astype(NPF32), ymin.astype(NPF32)


def _max_width(boxes_all):
    b = np.asarray(boxes_all, dtype=np.float64)
    valid = ~np.all(b == 0, axis=-1)
    bw = np.where(valid, b[..., 2] - b[..., 0], 1.0)
    bh = np.where(valid, b[..., 3] - b[..., 1], 1.0)
    ratio = np.where(bw > bh, bw / bh, bh / bw)
    ratio = np.where(valid, ratio, 0.0)
    return int(np.ceil(ratio.max() * PH))


def _grid_wide(xmin, ymin, bw, bh, wf, W, H, ii, jj):
    gx = ((xmin + (jj * bw / (wf - NPF32(1.0))).astype(NPF32)).astype(NPF32)
          - NPF32(W / 2)) / NPF32(W / 2)
    gy = ((ymin + (ii * bh / NPF32(PH - 1.0)).astype(NPF32)).astype(NPF32)
          - NPF32(H / 2)) / NPF32(H / 2)
    return gx.astype(NPF32), gy.astype(NPF32)


def _grid_tall(xmin, ymin, bw, bh, wf, W, H, ii, jj):
    gx = ((xmin + (ii * bw / NPF32(PH - 1.0)).astype(NPF32)).astype(NPF32)
          - NPF32(W / 2)) / NPF32(W / 2)
    gy = ((ymin + ((wf - jj) * bh / (wf - NPF32(1.0))).astype(NPF32)).astype(NPF32)
          - NPF32(H / 2)) / NPF32(H / 2)
    return gx.astype(NPF32), gy.astype(NPF32)


def _taps(g, n):
    g = g.astype(NPF32)
    pos = ((g + NPF32(1.0)) * NPF32(n) - NPF32(1.0)) * NPF32(0.5)
    pos64 = pos.astype(np.float64)
    i0 = np.floor(pos64).astype(np.int64)
    f = pos64 - i0
    w0 = np.where((i0 >= 0) & (i0 <= n - 1), 1.0 - f, 0.0)
    w1 = np.where((i0 + 1 >= 0) & (i0 + 1 <= n - 1), f, 0.0)
    return i0, w0, w1


def _build_image_weights(boxes, H, W, MW):
    K = boxes.shape[0]
    valid, wide, width, wf, bw, bh, xmin, ymin = _box_meta(boxes)
    WY = np.zeros((H, K * PH), np.float64)
    WXF = np.zeros((W, K * 2 * MW), np.float64)
    tall_idx = []
    ii = np.arange(PH, dtype=NPF32)
    for k in range(K):
        if not valid[k]:
            continue
        if not wide[k]:
            tall_idx.append(k)
            continue
        wk = int(width[k])
        jj = np.arange(wk, dtype=NPF32)
        gx, gy = _grid_wide(xmin[k], ymin[k], bw[k], bh[k], wf[k], W, H, ii, jj)
        y0, wy0, wy1 = _taps(gy, H)
        for i in range(PH):
            col = k * PH + i
            if wy0[i] != 0.0:
                WY[y0[i], col] += wy0[i]
            if wy1[i] != 0.0:
                WY[y0[i] + 1, col] += wy1[i]
        x0, wx0, wx1 = _taps(gx, W)
        base = k * 2 * MW
        for j in range(min(wk, MW)):
            if wx0[j] != 0.0:
                WXF[x0[j], base + j] += wx0[j]
            if wx1[j] != 0.0:
                WXF[x0[j] + 1, base + j] += wx1[j]
            jr = wk - 1 - j
            if wx0[jr] != 0.0:
                WXF[x0[jr], base + MW + j] += wx0[jr]
            if wx1[jr] != 0.0:
                WXF[x0[jr] + 1, base + MW + j] += wx1[jr]
    return WY.astype(NPF32), WXF.astype(NPF32), width, tall_idx


def _tall_feats(img, boxes, k, H, W, MW):
    valid, wide, width, wf, bw, bh, xmin, ymin = _box_meta(boxes)
    C = img.shape[0]
    wk = int(width[k])
    out = np.zeros((2, C, PH, MW), NPF32)
    ii = np.arange(PH, dtype=NPF32)[:, None]
    jj = np.arange(wk, dtype=NPF32)[None, :]
    gx, gy = _grid_tall(xmin[k], ymin[k], bw[k], bh[k], wf[k], W, H, ii, jj)
    gx = np.broadcast_to(gx, (PH, wk))
    gy = np.broadcast_to(gy, (PH, wk))
    x0, wx0, wx1 = _taps(gx, W)
    y0, wy0, wy1 = _taps(gy, H)
    imgf = img.astype(np.float64)

    def gat(yc, xc, m):
        yi = np.clip(yc, 0, H - 1)
        xi = np.clip(xc, 0, W - 1)
        return imgf[:, yi, xi] * m

    s = (gat(y0, x0, wy0 * wx0) + gat(y0, x0 + 1, wy0 * wx1)
         + gat(y0 + 1, x0, wy1 * wx0) + gat(y0 + 1, x0 + 1, wy1 * wx1))
    wcl = min(wk, MW)
    out[0, :, :, :wcl] = s[:, :, :wcl].astype(NPF32)
    out[1, :, :, :wcl] = s[:, ::-1, ::-1][:, :, :wcl].astype(NPF32)
    return out


# ----------------------------------------------------------------------------
# device program
# ----------------------------------------------------------------------------

S1_DTYPE = F16  # stage-1 matmul dtype: F16 (fast) or F32R (higher precision)


def _build_program(C, K, MW, KG=32, P=128, s1_dtype=None):
    s1_dtype = s1_dtype or S1_DTYPE
    NW = 2 * MW
    G = K // KG
    assert K % KG == 0
    ncols = KG * PH

    nc = bacc.Bacc("TRN2", target_bir_lowering=False, debug=False,
                   enable_asserts=True, num_devices=1)

    img = nc.dram_tensor("img", [P, C * P], s1_dtype, kind="ExternalInput").ap()
    wy = nc.dram_tensor("wy", [P, K * PH], s1_dtype, kind="ExternalInput").ap()
    wxf = nc.dram_tensor("wxf", [P, K * NW], F16, kind="ExternalInput").ap()
    feats = nc.dram_tensor("feats", [K, 2 * C * PH * MW], F32,
                           kind="ExternalOutput").ap()

    blk = PH * MW
    off1 = ((blk + 511) // 512) * 512  # dir1 at a PSUM-bank-aligned column
    ps2_cols = off1 + blk

    with tile.TileContext(nc) as tc, ExitStack() as ctx:
        const_pool = ctx.enter_context(tc.tile_pool(name="const", bufs=1))
        s_pool = ctx.enter_context(tc.tile_pool(name="sg", bufs=2))
        wxf_pool = ctx.enter_context(tc.tile_pool(name="wxf", bufs=1))
        out_pool = ctx.enter_context(tc.tile_pool(name="outt", bufs=4))
        ps1_pool = ctx.enter_context(tc.tile_pool(name="ps1", bufs=4, space="PSUM"))
        ps2_pool = ctx.enter_context(tc.tile_pool(name="ps2", bufs=2, space="PSUM"))

        img_t = const_pool.tile([P, C * P], s1_dtype)
        wy_t = const_pool.tile([P, K * PH], s1_dtype)
        nc.sync.dma_start(img_t[:], img)
        nc.sync.dma_start(wy_t[:], wy)

        for g in range(G):
            # S layout is c-major: free index = c*ncols + ki, so the
            # PSUM->SBUF casts write contiguously; stage-2 stationaries
            # read a strided AP instead. Casts split 2:1 over DVE:ACT.
            sg = s_pool.tile([P, C * ncols], F16, tag="sg")
            wxf_t = wxf_pool.tile([P, KG * NW], F16, tag="wxf")
            nc.sync.dma_start(wxf_t[:], wxf[:, g * KG * NW:(g + 1) * KG * NW])
            for c in range(C):
                ps1 = ps1_pool.tile([P, ncols], F32, tag="ps1")
                nc.tensor.matmul(
                    ps1[:],
                    img_t[:, c * P:(c + 1) * P],
                    wy_t[:, g * ncols:(g + 1) * ncols],
                )
                if c % 3 == 2:
                    nc.scalar.copy(sg[:, c * ncols:(c + 1) * ncols], ps1[:])
                else:
                    nc.vector.tensor_copy(
                        sg[:, c * ncols:(c + 1) * ncols], ps1[:])

            for kl in range(KG):
                k = g * KG + kl
                ps2 = ps2_pool.tile([C, ps2_cols], F32, tag="ps2")
                for i in range(PH):
                    lhsT = sg[:, kl * PH + i::ncols]  # [P, C] strided
                    nc.tensor.matmul(
                        ps2[:, i * MW:(i + 1) * MW],
                        lhsT, wxf_t[:, kl * NW:kl * NW + MW])
                    nc.tensor.matmul(
                        ps2[:, off1 + (PH - 1 - i) * MW:off1 + (PH - i) * MW],
                        lhsT, wxf_t[:, kl * NW + MW:kl * NW + 2 * MW])

                outt = out_pool.tile([C, 2 * blk], F32, tag="outt")
                if kl % 3 == 2:
                    nc.scalar.copy(outt[:, :blk], ps2[:, :blk])
                    nc.scalar.copy(outt[:, blk:], ps2[:, off1:off1 + blk])
                else:
                    nc.vector.tensor_copy(outt[:, :blk], ps2[:, :blk])
                    nc.vector.tensor_copy(outt[:, blk:], ps2[:, off1:off1 + blk])
                # DRAM box block order is (d, c, ij); SBUF iterates (c, d, ij)
                box = bass.AP(feats.tensor, k * 2 * C * blk,
                              [[blk, C], [C * blk, 2], [1, blk]])
                nc.sync.dma_start(box, outt[:])

    nc.compile()
    return nc


_PROGRAM_CACHE = {}


def _get_program(C, K, MW):
    key = (C, K, MW)
    if key not in _PROGRAM_CACHE:
        _PROGRAM_CACHE[key] = _build_program(C, K, MW)
    return _PROGRAM_CACHE[key]


# ----------------------------------------------------------------------------
# entry point
# ----------------------------------------------------------------------------

def kernel(x, boxes, _run_kwargs=None):
    x = np.asarray(x, dtype=np.float32)
    boxes = np.asarray(boxes, dtype=np.float32)
    N, C, H, W = x.shape
    K = boxes.shape[1]
    assert N == N_CORES and H == 128 and W == 128 and C == 128

    MW = _max_width(boxes)
    nc = _get_program(C, K, MW)

    s1_np = np.float16 if S1_DTYPE == F16 else np.float32
    in_maps = []
    per_image = []
    for n in range(N):
        WY, WXF, width, tall_idx = _build_image_weights(boxes[n], H, W, MW)
        per_image.append((width, tall_idx))
        img = np.ascontiguousarray(
            x[n].transpose(1, 0, 2).reshape(H, C * W))  # [h, (c, w)]
        in_maps.append({
            "img": img.astype(s1_np),
            "wy": WY.astype(s1_np),
            "wxf": WXF.astype(np.float16),
        })

    res = bass_utils.run_bass_kernel_spmd(
        nc, in_maps, core_ids=list(range(N_CORES)), **(_run_kwargs or {}))

    feats = np.empty((N, K, 2, C, PH, MW), np.float32)
    widths = np.empty((N, K, 2), np.float32)
    for n in range(N):
        feats[n] = res.results[n]["feats"].reshape(K, 2, C, PH, MW)
        width, tall_idx = per_image[n]
        for k in tall_idx:
            feats[n, k] = _tall_feats(x[n], boxes[n], k, H, W, MW)
        widths[n] = width.astype(np.float32)[:, None]
    kernel.last_result = res
    return feats, widths
